# revision 2
# baseline (speedup 1.0000x reference)
"""Trainium2 Bass kernel for stacked-head state attention (nn_ARCStacked).

Problem (hardcoded shapes):
  x: (2, 2304, 2048) f32; six weights (16, 2048, 128) f32; scaling (16,) f32.
  T = 2304 = 128 state + 2048 core + 128 state tokens. Per (batch b, head h):
    q/k/v = l2norm(x @ w[h]) per token (state weights for the two 128-token
    state segments, core weights for the middle 2048), causal attention with
    the extra rule that the last 128 rows can't attend to the first 128 cols,
    out = softmax(scale_h * q @ k^T + mask) @ v  -> (2, 16, 2304, 128) f32.

Sharding: 8 cores = 2 batches x 4 head-groups (4 heads each). Outputs are
disjoint -> no collectives. Per core:
  Phase A: token-major projections for 4 heads at once (rhs = [d, 4*128]
  stacked weights), per-token l2norm on the free axis (ACT square+accum,
  DVE reciprocal), PE-transpose of q/k into dk-major layout for attention.
  q's norm factor also folds in scale_h.
  Phase B: per head, per 128-token q-tile: transposed scores block
  s^T[kt, qt] = k^T_slice.T @ q^T_slice on PE, exp on ACT (no max-subtraction
  needed: q,k are unit vectors so |s| <= scale), diagonal blocks masked by an
  upper-triangular 0/1 mask, then attn@v via lhsT = exp-block with a ones
  column appended to v giving the softmax denominator in the same PSUM
  accumulation. Final divide + DMA out.
"""

import numpy as np
import ml_dtypes

B = 2
H = 16
T = 2304
D = 2048
DK = 128
NT = T // 128  # 18 token tiles; tile 0 and 17 are state segments
NCH = D // 128  # 16 contraction chunks
HG = 4  # heads per core
N_CORES = 8

_NC = None


def build_nc():
    """Build (once) the single-core Bass graph shared by all 8 cores."""
    global _NC
    if _NC is not None:
        return _NC

    from contextlib import ExitStack

    import concourse.tile as tile
    from concourse import bacc, mybir
    from concourse.masks import make_identity, make_upper_triangular

    bf16 = mybir.dt.bfloat16
    f32 = mybir.dt.float32
    AF = mybir.ActivationFunctionType

    nc = bacc.Bacc()
    xt = nc.declare_dram_parameter("xt", [128, NT, NCH, 128], bf16, isOutput=False)
    w_params = {
        name: nc.declare_dram_parameter(name, [128, NCH, HG * 128], bf16, isOutput=False)
        for name in ("wq", "wk", "wv", "wqs", "wks", "wvs")
    }
    scal = nc.declare_dram_parameter("scal", [128, HG], f32, isOutput=False)
    out = nc.declare_dram_parameter("out", [HG, T, DK], f32, isOutput=True)

    with tile.TileContext(nc) as tc, ExitStack() as ctx:
        consts = ctx.enter_context(tc.tile_pool(name="consts", bufs=1))
        identity = consts.tile([128, 128], bf16)
        make_identity(nc, identity[:])
        # triT[i, j] = 1 iff i <= j : allowed entries of a transposed-scores
        # diagonal block (kt_local <= qt_local)
        triT = consts.tile([128, 128], bf16)
        make_upper_triangular(nc, triT[:], val=1.0, diag=True)
        scal_s = consts.tile([128, HG], f32)
        nc.sync.dma_start(scal_s[:], scal[:])

        wpool = ctx.enter_context(tc.tile_pool(name="w", bufs=1))
        w_s = {}
        for name, p in w_params.items():
            t = wpool.tile([128, NCH, HG * 128], bf16, tag=name)
            nc.sync.dma_start(t[:], p[:])
            w_s[name] = t

        big = ctx.enter_context(tc.tile_pool(name="big", bufs=1))
        qT_s = big.tile([128, HG, T], bf16, tag="qT")   # [dk, h, t]
        kT_s = big.tile([128, HG, T], bf16, tag="kT")   # [dk, h, t]
        v_s = big.tile([128, HG, NT, 129], bf16, tag="v")  # [t_local, h, ti, dv+1]
        nc.vector.memset(v_s[:, :, :, 128:129], 1.0)

        xpool = ctx.enter_context(tc.tile_pool(name="x", bufs=3))

        # ---------------- Phase A: projections + l2norm + transposes --------
        with (
            tc.tile_pool(name="psA", bufs=4, space="PSUM") as psA,
            tc.tile_pool(name="psT", bufs=3, space="PSUM") as psT,
            tc.tile_pool(name="wkA", bufs=3) as wkA,
        ):
            for i in range(NT):
                xs = xpool.tile([128, NCH, 128], bf16)
                nc.sync.dma_start(xs[:], xt[:, i])
                state = i == 0 or i == NT - 1
                for kind, wc, ws in (("q", "wq", "wqs"), ("k", "wk", "wks"),
                                     ("v", "wv", "wvs")):
                    w_t = w_s[ws] if state else w_s[wc]
                    ps = psA.tile([128, HG * 128], f32, tag="proj")
                    for c in range(NCH):
                        nc.tensor.matmul(
                            ps[:], xs[:, c, :], w_t[:, c, :],
                            start=(c == 0), stop=(c == NCH - 1),
                        )
                    # per-token 1/||.|| (free-axis reduction per 128-wide head)
                    sq = wkA.tile([128, HG * 128], f32, tag="sq")
                    n2 = wkA.tile([128, HG], f32, tag="n2")
                    for h in range(HG):
                        nc.scalar.activation(
                            sq[:, h * 128:(h + 1) * 128],
                            ps[:, h * 128:(h + 1) * 128],
                            AF.Square,
                            accum_out=n2[:, h:h + 1],
                        )
                    nrm = wkA.tile([128, HG], f32, tag="nrm")
                    nc.scalar.sqrt(nrm[:], n2[:])
                    fac = wkA.tile([128, HG], f32, tag="fac")
                    nc.vector.reciprocal(fac[:], nrm[:])
                    if kind == "q":
                        facq = wkA.tile([128, HG], f32, tag="facq")
                        nc.vector.tensor_mul(facq[:], fac[:], scal_s[:])
                        fac = facq
                    if kind == "v":
                        for h in range(HG):
                            nc.vector.tensor_scalar_mul(
                                v_s[:, h, i, 0:128],
                                ps[:, h * 128:(h + 1) * 128],
                                fac[:, h:h + 1],
                            )
                    else:
                        nrmd = wkA.tile([128, HG * 128], bf16, tag="nrmd")
                        for h in range(HG):
                            nc.vector.tensor_scalar_mul(
                                nrmd[:, h * 128:(h + 1) * 128],
                                ps[:, h * 128:(h + 1) * 128],
                                fac[:, h:h + 1],
                            )
                        dest = qT_s if kind == "q" else kT_s
                        for h in range(HG):
                            pt = psT.tile([128, 128], bf16, tag="tp")
                            nc.tensor.transpose(
                                pt[:], nrmd[:, h * 128:(h + 1) * 128], identity[:]
                            )
                            nc.vector.tensor_copy(
                                dest[:, h, i * 128:(i + 1) * 128], pt[:]
                            )

        # ---------------- Phase B: attention per head -----------------------
        with (
            tc.tile_pool(name="psS", bufs=4, space="PSUM") as psS,
            tc.tile_pool(name="psO", bufs=2, space="PSUM") as psO,
            tc.tile_pool(name="wkB", bufs=6) as wkB,
            tc.tile_pool(name="outp", bufs=3) as outp,
        ):
            for h in range(HG):
                for qi in range(NT):
                    po = psO.tile([128, 129], f32, tag="po")
                    kis = [ki for ki in range(qi + 1)
                           if not (qi == NT - 1 and ki == 0)]
                    for idx, ki in enumerate(kis):
                        ps = psS.tile([128, 128], f32, tag="sc")
                        nc.tensor.matmul(
                            ps[:],
                            kT_s[:, h, ki * 128:(ki + 1) * 128],
                            qT_s[:, h, qi * 128:(qi + 1) * 128],
                            start=True, stop=True,
                        )
                        ex = wkB.tile([128, 128], bf16, tag="ex")
                        nc.scalar.activation(ex[:], ps[:], AF.Exp)
                        if ki == qi:
                            nc.vector.tensor_mul(ex[:], ex[:], triT[:])
                        nc.tensor.matmul(
                            po[:], ex[:], v_s[:, h, ki, :],
                            start=(idx == 0), stop=(idx == len(kis) - 1),
                        )
                    den = wkB.tile([128, 1], f32, tag="den")
                    nc.vector.reciprocal(den[:], po[:, 128:129])
                    ot = outp.tile([128, 128], f32, tag="ot")
                    nc.vector.tensor_scalar_mul(ot[:], po[:, 0:128], den[:])
                    nc.sync.dma_start(out[h, qi * 128:(qi + 1) * 128, :], ot[:])

    nc.finalize()
    _NC = nc
    return nc


def _shard_inputs(x, w_q, w_k, w_v, w_q_state, w_k_state, w_v_state,
                  scaling_factor):
    bf16 = ml_dtypes.bfloat16

    def prep_x(xb):
        # (T, D) -> [128 part=d%128, NT, NCH, 128] so each token-tile slice is
        # contiguous per partition
        xt = np.ascontiguousarray(xb.T)                      # (D, T)
        xt = xt.reshape(NCH, 128, NT, 128).transpose(1, 2, 0, 3)
        return np.ascontiguousarray(xt.astype(bf16))

    def prep_w(w, g):
        # (H, D, DK) -> heads 4g..4g+3 stacked on the free axis ->
        # [128 part=d%128, NCH, 4*128]
        w4 = w[HG * g:HG * (g + 1)].transpose(1, 0, 2).reshape(D, HG * DK)
        w4 = w4.reshape(NCH, 128, HG * DK).transpose(1, 0, 2)
        return np.ascontiguousarray(w4.astype(bf16))

    xts = [prep_x(np.asarray(x[b], dtype=np.float32)) for b in range(B)]
    in_maps = []
    for core in range(N_CORES):
        b, g = divmod(core, N_CORES // B)
        m = {"xt": xts[b]}
        for name, w in (("wq", w_q), ("wk", w_k), ("wv", w_v),
                        ("wqs", w_q_state), ("wks", w_k_state),
                        ("wvs", w_v_state)):
            m[name] = prep_w(np.asarray(w, dtype=np.float32), g)
        sc = np.asarray(scaling_factor, dtype=np.float32)[HG * g:HG * (g + 1)]
        m["scal"] = np.ascontiguousarray(
            np.broadcast_to(sc[None, :], (128, HG)).astype(np.float32))
        in_maps.append(m)
    return in_maps


def run_on_cores(in_maps, **kwargs):
    from concourse.bass_utils import run_bass_kernel_spmd

    nc = build_nc()
    return run_bass_kernel_spmd(nc, in_maps, list(range(N_CORES)), **kwargs)


def kernel(x, w_q, w_k, w_v, w_q_state, w_k_state, w_v_state, scaling_factor):
    in_maps = _shard_inputs(x, w_q, w_k, w_v, w_q_state, w_k_state, w_v_state,
                            scaling_factor)
    res = run_on_cores(in_maps)
    full = np.empty((B, H, T, DK), dtype=np.float32)
    for core in range(N_CORES):
        b, g = divmod(core, N_CORES // B)
        full[b, HG * g:HG * (g + 1)] = res.results[core]["out"]
    return full


# revision 4
# speedup vs baseline: 1.0427x; 1.0427x over previous
"""Trainium2 Bass kernel for stacked-head state attention (nn_ARCStacked).

Problem (hardcoded shapes):
  x: (2, 2304, 2048) f32; six weights (16, 2048, 128) f32; scaling (16,) f32.
  T = 2304 = 128 state + 2048 core + 128 state tokens. Per (batch b, head h):
    q/k/v = l2norm(x @ w[h]) per token (state weights for the two 128-token
    state segments, core weights for the middle 2048), causal attention with
    the extra rule that the last 128 rows can't attend to the first 128 cols,
    out = softmax(scale_h * q @ k^T + mask) @ v  -> (2, 16, 2304, 128) f32.

Sharding: 8 cores = 2 batches x 4 head-groups (4 heads each); outputs are
disjoint -> no collectives.

Per-core structure (v2): loop over five 512-token chunks J (last is 256).
  A_qk(J): q^T/k^T projected directly in [dk, token] layout (stationary = the
    head's weight chunk, moving = x^T), L2 norms via an all-ones [128,128]
    stationary matmul on the squared values (broadcast column sums), so the
    per-token normalizer never needs a partition reduction on DVE.
  A_v(J): v in token-major layout (norm on the free axis via ACT square+accum),
    stored with a ones column appended -> attn @ [v|1] yields the softmax
    denominator inside the same PSUM accumulation.
  B(J): attention for q-tiles of J (causal: only needs k/v tiles <= J). Scores
    are computed transposed (s^T[kt, qt]) so the exp blocks feed attn@v as the
    stationary operand directly. |s| <= scale (unit vectors) -> no max
    subtraction. scale_h applied via the Exp activation's scale argument.
    4 consecutive kt-blocks pack one PSUM bank -> one [128,<=512] Exp per group.
"""

import numpy as np
import ml_dtypes

B = 2
H = 16
T = 2304
D = 2048
DK = 128
NT = T // 128   # 18 token tiles; tiles 0 and 17 are the state segments
NCH = D // 128  # 16 contraction chunks
HG = 4          # heads per core
N_CORES = 8
NJ = 5          # 512-token chunks (last is 256)

_NC = None


def build_nc():
    global _NC
    if _NC is not None:
        return _NC

    from contextlib import ExitStack

    import concourse.tile as tile
    from concourse import bacc, mybir
    from concourse.masks import make_upper_triangular

    bf16 = mybir.dt.bfloat16
    f32 = mybir.dt.float32
    AF = mybir.ActivationFunctionType

    nc = bacc.Bacc()
    xt = nc.declare_dram_parameter("xt", [128, NT, NCH, 128], bf16, isOutput=False)
    w_params = {
        name: nc.declare_dram_parameter(name, [128, NCH, HG * 128], bf16,
                                        isOutput=False)
        for name in ("wq", "wk", "wv", "wqs", "wks", "wvs")
    }
    scal = nc.declare_dram_parameter("scal", [128, HG], f32, isOutput=False)
    out = nc.declare_dram_parameter("out", [HG, T, DK], f32, isOutput=True)

    with tile.TileContext(nc) as tc, ExitStack() as ctx:
        consts = ctx.enter_context(tc.tile_pool(name="consts", bufs=1))
        # triT[i, j] = 1 iff i <= j: allowed entries of a transposed-scores
        # diagonal block (kt_local <= qt_local)
        triT = consts.tile([128, 128], bf16)
        make_upper_triangular(nc, triT[:], val=1.0, diag=True)
        ones128 = consts.tile([128, 128], bf16)
        nc.vector.memset(ones128[:], 1.0)
        scal_s = consts.tile([128, HG], f32)
        nc.sync.dma_start(scal_s[:], scal[:])

        wpool = ctx.enter_context(tc.tile_pool(name="w", bufs=1))
        w_s = {}
        for name, p in w_params.items():
            t = wpool.tile([128, NCH, HG * 128], bf16, tag=name)
            nc.sync.dma_start(t[:], p[:])
            w_s[name] = t

        big = ctx.enter_context(tc.tile_pool(name="big", bufs=1))
        kT_s = big.tile([128, HG, T], bf16, tag="kT")          # [dk, h, t]
        v_s = big.tile([128, HG, NT, 129], bf16, tag="v")      # [tl, h, ti, dv|1]
        nc.vector.memset(v_s[:, :, :, 128:129], 1.0)

        xpool = ctx.enter_context(tc.tile_pool(name="x", bufs=2))
        qpool = ctx.enter_context(tc.tile_pool(name="q", bufs=2))

        with (
            tc.tile_pool(name="psA", bufs=3, space="PSUM") as psA,
            tc.tile_pool(name="misc", bufs=3, space="PSUM") as misc,
            tc.tile_pool(name="psO", bufs=2, space="PSUM") as psO,
            tc.tile_pool(name="wk", bufs=3) as wk,
            tc.tile_pool(name="exw", bufs=4) as exw,
            tc.tile_pool(name="outp", bufs=3) as outp,
        ):
            for J in range(NJ):
                ntj = 4 if J < 4 else 2
                W = ntj * 128
                # x chunk: [128=d%128, tile, c, tl]
                xs = xpool.tile([128, ntj, NCH, 128], bf16, tag="xs")
                nc.sync.dma_start(xs[:], xt[:, 4 * J:4 * J + ntj])
                qT_j = qpool.tile([128, HG, 512], bf16, tag="qT")

                state_w = {0: True, NJ - 1: True}.get(J, False)
                # tiles 0 and 17 are state; J=0 has tiles 0..3 (mixed), J=4 has
                # 16,17 (mixed) -> choose weights per token tile for v; for q/k
                # the 512-wide matmul spans tiles with different weights, so
                # split the contraction per token tile instead: do matmuls per
                # (c, tile-group of same kind).

                # ---- A_qk: transposed projections + matmul-based norms ----
                for kind, wc, ws, dest in (("q", "wq", "wqs", None),
                                           ("k", "wk", "wks", None)):
                    for h in range(HG):
                        ps = psA.tile([128, 512], f32, tag="proj")
                        first = True
                        # group token tiles by weight kind within the chunk
                        for c in range(NCH):
                            for lo, hi, wname in _wgroups(J, ntj, wc, ws):
                                nc.tensor.matmul(
                                    ps[:, lo * 128:hi * 128],
                                    w_s[wname][:, c, h * 128:(h + 1) * 128],
                                    xs[:, lo:hi, c, :],
                                    start=(c == 0), stop=(c == NCH - 1),
                                )
                        sq = wk.tile([128, 512], bf16, tag="sq")
                        nc.scalar.activation(sq[:, :W], ps[:, :W], AF.Square)
                        n2 = misc.tile([128, 512], f32, tag="mb")
                        nc.tensor.matmul(n2[:, :W], ones128[:], sq[:, :W],
                                         start=True, stop=True)
                        sn = wk.tile([128, 512], f32, tag="sn")
                        nc.scalar.sqrt(sn[:, :W], n2[:, :W])
                        nc.vector.reciprocal(sn[:, :W], sn[:, :W])
                        tgt = (qT_j[:, h, :W] if kind == "q"
                               else kT_s[:, h, 512 * J:512 * J + W])
                        nc.vector.tensor_mul(tgt, ps[:, :W], sn[:, :W])

                # ---- A_v: token-major v + ones column ----
                for tl in range(ntj):
                    i = 4 * J + tl
                    wname = "wvs" if i in (0, NT - 1) else "wv"
                    pv = psA.tile([128, 512], f32, tag="proj")
                    for c in range(NCH):
                        nc.tensor.matmul(
                            pv[:], xs[:, tl, c, :], w_s[wname][:, c, :],
                            start=(c == 0), stop=(c == NCH - 1),
                        )
                    sqv = wk.tile([128, 512], bf16, tag="sqv")
                    n2v = wk.tile([128, HG], f32, tag="n2v")
                    for h in range(HG):
                        nc.scalar.activation(
                            sqv[:, h * 128:(h + 1) * 128],
                            pv[:, h * 128:(h + 1) * 128],
                            AF.Square, accum_out=n2v[:, h:h + 1],
                        )
                    nc.scalar.sqrt(n2v[:], n2v[:])
                    nc.vector.reciprocal(n2v[:], n2v[:])
                    for h in range(HG):
                        nc.vector.tensor_scalar_mul(
                            v_s[:, h, i, 0:128],
                            pv[:, h * 128:(h + 1) * 128],
                            n2v[:, h:h + 1],
                        )

                # ---- B: attention for q-tiles of this chunk ----
                for h in range(HG):
                    for ql in range(ntj):
                        qi = 4 * J + ql
                        po = psO.tile([128, 129], f32, tag="po")
                        kis = [ki for ki in range(qi + 1)
                               if not (qi == NT - 1 and ki == 0)]
                        ngroups = qi // 4 + 1
                        first_av = True
                        for g in range(ngroups):
                            gkis = [ki for ki in range(4 * g, min(4 * g + 4, qi + 1))]
                            gw = len(gkis) * 128
                            pss = misc.tile([128, 512], f32, tag="mb")
                            for sl, ki in enumerate(gkis):
                                nc.tensor.matmul(
                                    pss[:, sl * 128:(sl + 1) * 128],
                                    kT_s[:, h, ki * 128:(ki + 1) * 128],
                                    qT_j[:, h, ql * 128:(ql + 1) * 128],
                                    start=True, stop=True,
                                )
                            ex = exw.tile([128, 512], bf16, tag="ex")
                            nc.scalar.activation(ex[:, :gw], pss[:, :gw], AF.Exp,
                                                 scale=scal_s[:, h:h + 1])
                            if gkis[-1] == qi:
                                sl = len(gkis) - 1
                                nc.vector.tensor_mul(
                                    ex[:, sl * 128:(sl + 1) * 128],
                                    ex[:, sl * 128:(sl + 1) * 128],
                                    triT[:],
                                )
                            for sl, ki in enumerate(gkis):
                                if ki not in kis:
                                    continue
                                nc.tensor.matmul(
                                    po[:], ex[:, sl * 128:(sl + 1) * 128],
                                    v_s[:, h, ki, :],
                                    start=first_av, stop=(ki == kis[-1]),
                                )
                                first_av = False
                        den = exw.tile([128, 1], f32, tag="den")
                        nc.vector.reciprocal(den[:], po[:, 128:129])
                        ot = outp.tile([128, 128], f32, tag="ot")
                        nc.vector.tensor_scalar_mul(ot[:], po[:, 0:128], den[:])
                        nc.sync.dma_start(out[h, qi * 128:(qi + 1) * 128, :], ot[:])

    nc.finalize()
    _NC = nc
    return nc


def _wgroups(J, ntj, wc, ws):
    """Token-tile ranges [lo, hi) within chunk J sharing one weight tensor."""
    if J == 0:
        return [(0, 1, ws), (1, ntj, wc)]
    if J == NJ - 1:
        return [(0, ntj - 1, wc), (ntj - 1, ntj, ws)]
    return [(0, ntj, wc)]


def _shard_inputs(x, w_q, w_k, w_v, w_q_state, w_k_state, w_v_state,
                  scaling_factor):
    bf16 = ml_dtypes.bfloat16

    def prep_x(xb):
        xt = np.ascontiguousarray(xb.T)                       # (D, T)
        xt = xt.reshape(NCH, 128, NT, 128).transpose(1, 2, 0, 3)
        return np.ascontiguousarray(xt.astype(bf16))

    def prep_w(w, g):
        w4 = w[HG * g:HG * (g + 1)].transpose(1, 0, 2).reshape(D, HG * DK)
        w4 = w4.reshape(NCH, 128, HG * DK).transpose(1, 0, 2)
        return np.ascontiguousarray(w4.astype(bf16))

    xts = [prep_x(np.asarray(x[b], dtype=np.float32)) for b in range(B)]
    in_maps = []
    for core in range(N_CORES):
        b, g = divmod(core, N_CORES // B)
        m = {"xt": xts[b]}
        for name, w in (("wq", w_q), ("wk", w_k), ("wv", w_v),
                        ("wqs", w_q_state), ("wks", w_k_state),
                        ("wvs", w_v_state)):
            m[name] = prep_w(np.asarray(w, dtype=np.float32), g)
        sc = np.asarray(scaling_factor, dtype=np.float32)[HG * g:HG * (g + 1)]
        m["scal"] = np.ascontiguousarray(
            np.broadcast_to(sc[None, :], (128, HG)).astype(np.float32))
        in_maps.append(m)
    return in_maps


def run_on_cores(in_maps, **kwargs):
    from concourse.bass_utils import run_bass_kernel_spmd

    nc = build_nc()
    return run_bass_kernel_spmd(nc, in_maps, list(range(N_CORES)), **kwargs)


def kernel(x, w_q, w_k, w_v, w_q_state, w_k_state, w_v_state, scaling_factor):
    in_maps = _shard_inputs(x, w_q, w_k, w_v, w_q_state, w_k_state, w_v_state,
                            scaling_factor)
    res = run_on_cores(in_maps)
    full = np.empty((B, H, T, DK), dtype=np.float32)
    for core in range(N_CORES):
        b, g = divmod(core, N_CORES // B)
        full[b, HG * g:HG * (g + 1)] = res.results[core]["out"]
    return full
